# revision 16
# baseline (speedup 1.0000x reference)
"""Llama4TextExperts MoE kernel for 8 Trainium2 NeuronCores — v2 (bf16).

Expert-parallel: core e handles expert e (tokens pre-sorted per expert).
Per core: x_e (1024,2048) @ gate_up[e] (2048,8192) -> silu(gate)*up ->
@ down[e] (4096,2048) -> out_e (1024,2048).

v2 changes vs the fp32r baseline:
- All matmuls in bf16 (rel err ~4e-3, well under the 2e-2 gate). bf16
  halves weight DMA (96 -> 48 MB/core) and makes the acted tensor small
  enough (8 MB) to stay SBUF-resident, removing the 32 MB DRAM spill
  round-trip and the phase-transition PE bubble.
- Tile legalization emits one LDWEIGHTS per matmul even when consecutive
  matmuls share a weight tile; a post-pass removes the redundant LDW of
  each (gate|up, kk) pair so each weight load serves both 512-token
  chunks. Sync info of removed LDWs moves to the following instruction
  (later = safe).
- Single flat tile region so the PE queue flows phase1 -> phase2 with no
  barrier; PSUM tags are shared between phases to stay within 8 banks.
"""

import numpy as np

NUM_EXPERTS = 8
HIDDEN = 2048
INTER = 4096
TOKENS = 8192
T = TOKENS // NUM_EXPERTS  # 1024 tokens per expert/core
TK = HIDDEN // 128  # 16 contraction tiles in phase 1
TI = INTER // 128  # 32 feature tiles of gate/up; contraction tiles in phase 2
TH = HIDDEN // 128  # 16 output feature tiles
NT = T // 512  # 2 token chunks of 512


def _split_waits(nc, max_waits=1):
    """The walrus build in this environment rejects instructions carrying
    more than one sync wait. Move excess SyncWaits onto preceding NoOps
    on the same engine."""
    import concourse.mybir as mybir

    for fn in nc.m.functions:
        for blk in fn.blocks:
            new_insts = []
            for inst in blk.instructions:
                si = inst.sync_info
                if si is not None and len(si.on_wait) > max_waits:
                    waits = list(si.on_wait)
                    excess, keep = waits[:-max_waits], waits[-max_waits:]
                    for i in range(0, len(excess), max_waits):
                        chunk = excess[i : i + max_waits]
                        new_insts.append(
                            mybir.InstNoOp(
                                name=f"{inst.name}-waitsplit-{i}",
                                ins=[],
                                outs=[],
                                engine=inst.engine,
                                sync_info=mybir.SyncInfo(
                                    on_wait=list(chunk), on_update=[]
                                ),
                            )
                        )
                    si.on_wait = keep
                new_insts.append(inst)
            blk.instructions = new_insts


def _dedup_ldweights(nc):
    """Remove InstLdweights whose weights AP equals the one already loaded
    (Tile emits 1:1 LDW:MM). Waits/updates of a removed LDW attach to the
    next instruction: both then take effect later than the original LDW
    would have, which is always safe."""
    import concourse.mybir as mybir

    removed = 0
    for fn in nc.m.functions:
        for blk in fn.blocks:
            loaded = None
            pending = []
            final = []
            for inst in blk.instructions:
                if isinstance(inst, mybir.InstLdweights):
                    key = repr(inst.ins[0])
                    if loaded == key and inst.perf_mode is None:
                        si = inst.sync_info
                        if si is not None:
                            pending.append(
                                (list(si.on_wait), list(si.on_update))
                            )
                        removed += 1
                        continue
                    loaded = key
                elif isinstance(inst, mybir.InstMatmult) and inst.is_transpose:
                    loaded = None
                if pending:
                    si = inst.sync_info
                    if si is None:
                        inst.sync_info = mybir.SyncInfo(on_wait=[], on_update=[])
                        si = inst.sync_info
                    for pw, pu in pending:
                        si.on_wait = list(si.on_wait) + pw
                        si.on_update = list(si.on_update) + pu
                    pending = []
                final.append(inst)
            blk.instructions = final
    return removed


def build_bass(repeat=1, postpasses=True, probe=None, dedup=True):
    import contextlib

    import concourse.bass as bass
    import concourse.mybir as mybir
    import concourse.tile as tile

    F32 = mybir.dt.float32
    BF16 = mybir.dt.bfloat16
    Silu = mybir.ActivationFunctionType.Silu

    nc = bass.Bass()
    xT = nc.declare_dram_parameter("xT", [HIDDEN, T], BF16, isOutput=False)
    # host-reordered tile-major: w1[g, p, kk, f] = gate_up[kk*128+p, g*128+f]
    # (g 0..31 = gate blocks, 32..63 = up blocks); w2[h, p, ii, f] = down[ii*128+p, h*128+f]
    w1 = nc.declare_dram_parameter("w1", [2 * TI, 128, TK, 128], BF16, isOutput=False)
    w2 = nc.declare_dram_parameter("w2", [TH, 128, TI, 128], BF16, isOutput=False)
    outT = nc.declare_dram_parameter("outT", [HIDDEN, T], F32, isOutput=True)

    xT_t = xT.rearrange("(kk p) t -> kk p t", p=128)
    outT_t = outT.rearrange("(hh p) t -> hh p t", p=128)

    with tile.TileContext(nc) as tc:
        rep = tc.For_i(0, repeat, 1) if repeat > 1 else contextlib.nullcontext()
        with rep:
            with tc.tile_pool(name="xres", bufs=1) as xp, \
                 tc.tile_pool(name="w1s", bufs=3) as w1p, \
                 tc.tile_pool(name="tmp", bufs=8) as tmpp, \
                 tc.tile_pool(name="acted", bufs=1) as actp, \
                 tc.tile_pool(name="w2s", bufs=3) as w2p, \
                 tc.tile_pool(name="outs", bufs=4) as outp, \
                 tc.tile_pool(name="ps", bufs=2, space="PSUM") as psp:
                xts = [
                    xp.tile([128, T], BF16, tag=f"x{k}", name=f"xres{k}")
                    for k in range(TK)
                ]
                for k in range(TK):
                    nc.sync.dma_start(out=xts[k], in_=xT_t[k])

                actts = (
                    [
                        actp.tile([128, T], BF16, tag=f"a{i}", name=f"acted{i}")
                        for i in range(TI)
                    ]
                    if probe not in ("noact", "mmonly")
                    else None
                )

                # probe="mmonly": flat stream of 3072 N=512 matmuls, shared
                # weights, 2 psum banks, two long accumulation groups — no
                # consumers, no bank cycling. Isolates the raw MM issue rate.
                if probe == "mmonly":
                    w_sh = w1p.tile([128, TK, 128], BF16, tag="wg")
                    nc.sync.dma_start(out=w_sh, in_=w1[0])
                    ps = [
                        psp.tile([128, 512], F32, tag=f"pg{t}", name=f"psmm{t}")
                        for t in range(2)
                    ]
                    NMM = 3072
                    for i in range(NMM):
                        b = i % 2
                        nc.tensor.matmul(
                            ps[b],
                            w_sh[:, 0, :],
                            xts[i % TK][:, (i // TK % NT) * 512:(i // TK % NT) * 512 + 512],
                            start=(i < 2),
                            stop=(i >= NMM - 2),
                        )
                    ot = outp.tile([128, T], F32, tag="ot")
                    for t in range(NT):
                        nc.vector.tensor_copy(ot[:, t * 512:(t + 1) * 512], ps[t])
                    nc.sync.dma_start(out=outT_t[0], in_=ot)

                # probe="noldw": single shared weight tile for every matmul so
                # the dedup pass strips all but ~1 LDWEIGHTS — isolates the
                # serialized LDW cost on HW (output is garbage; timing only).
                if probe == "noldw":
                    wg_sh = w1p.tile([128, TK, 128], BF16, tag="wg")
                    wu_sh = wg_sh
                    nc.sync.dma_start(out=wg_sh, in_=w1[0])

                # ---- Phase 1: gate/up projection + silu(gate)*up -> acted ----
                for g in range(TI if probe != "mmonly" else 0):
                    if probe == "noldw":
                        wg, wu = wg_sh, wu_sh
                    else:
                        wg = w1p.tile([128, TK, 128], BF16, tag="wg")
                        wu = w1p.tile([128, TK, 128], BF16, tag="wu")
                        nc.sync.dma_start(out=wg, in_=w1[g])
                        nc.sync.dma_start(out=wu, in_=w1[TI + g])
                    pg = [
                        psp.tile([128, 512], F32, tag=f"pg{t}", name=f"psg{g}_{t}")
                        for t in range(NT)
                    ]
                    pu = [
                        psp.tile([128, 512], F32, tag=f"pu{t}", name=f"psu{g}_{t}")
                        for t in range(NT)
                    ]
                    for kk in range(TK):
                        st, sp = kk == 0, kk == TK - 1
                        wk = 0 if probe == "noldw" else kk
                        for t in range(NT):
                            ts = slice(t * 512, (t + 1) * 512)
                            nc.tensor.matmul(
                                pg[t], wg[:, wk, :], xts[kk][:, ts], start=st, stop=sp
                            )
                        for t in range(NT):
                            ts = slice(t * 512, (t + 1) * 512)
                            nc.tensor.matmul(
                                pu[t], wu[:, wk, :], xts[kk][:, ts], start=st, stop=sp
                            )
                    if probe != "noact":
                        for t in range(NT):
                            ts = slice(t * 512, (t + 1) * 512)
                            sg = tmpp.tile([128, 512], F32, tag="sg")
                            nc.scalar.activation(sg, pg[t], Silu)
                            nc.vector.tensor_mul(actts[g][:, ts], sg, pu[t])

                # ---- Phase 2: down projection -> outT ----
                if probe == "noldw":
                    w2_sh = w2p.tile([128, TI, 128], BF16, tag="w2")
                    nc.sync.dma_start(out=w2_sh, in_=w2[0])
                for h in range(TH if probe != "mmonly" else 0):
                    if probe == "noldw":
                        w2h = w2_sh
                    else:
                        w2h = w2p.tile([128, TI, 128], BF16, tag="w2")
                        nc.sync.dma_start(out=w2h, in_=w2[h])
                    po = [
                        psp.tile([128, 512], F32, tag=f"pg{t}", name=f"pso{h}_{t}")
                        for t in range(NT)
                    ]
                    for ii in range(TI):
                        st, sp = ii == 0, ii == TI - 1
                        wi = 0 if probe == "noldw" else ii
                        mov = xts[ii % TK] if probe == "noact" else actts[ii]
                        for t in range(NT):
                            ts = slice(t * 512, (t + 1) * 512)
                            nc.tensor.matmul(
                                po[t], w2h[:, wi, :], mov[:, ts],
                                start=st, stop=sp,
                            )
                    if probe != "noact":
                        ot = outp.tile([128, T], F32, tag="ot")
                        for t in range(NT):
                            ts = slice(t * 512, (t + 1) * 512)
                            nc.vector.tensor_copy(ot[:, ts], po[t])
                        nc.sync.dma_start(out=outT_t[h], in_=ot)

    if postpasses:
        if dedup:
            _dedup_ldweights(nc)
        _split_waits(nc, 1)
    return nc


def make_in_maps(hidden_states, gate_up_proj, down_proj):
    import ml_dtypes

    BF = ml_dtypes.bfloat16
    x = np.asarray(hidden_states, dtype=np.float32).reshape(NUM_EXPERTS, T, HIDDEN)
    w1 = np.asarray(gate_up_proj, dtype=np.float32)
    w2 = np.asarray(down_proj, dtype=np.float32)
    in_maps = []
    for e in range(NUM_EXPERTS):
        # (H, 2I) -> (2I/128 g, 128 p, H/128 kk, 128 f) tile-major contiguous
        w1r = w1[e].reshape(TK, 128, 2 * TI, 128).transpose(2, 1, 0, 3)
        # (I, H) -> (H/128 h, 128 p, I/128 ii, 128 f)
        w2r = w2[e].reshape(TI, 128, TH, 128).transpose(2, 1, 0, 3)
        in_maps.append(
            {
                "xT": np.ascontiguousarray(x[e].T).astype(BF),
                "w1": np.ascontiguousarray(w1r).astype(BF),
                "w2": np.ascontiguousarray(w2r).astype(BF),
            }
        )
    return in_maps


def assemble_output(results):
    outs = [results[e]["outT"].T for e in range(NUM_EXPERTS)]
    return np.concatenate(outs, axis=0).astype(np.float32)


def kernel(hidden_states, gate_up_proj, down_proj):
    from concourse.bass_utils import run_bass_kernel_spmd

    nc = build_bass()
    in_maps = make_in_maps(hidden_states, gate_up_proj, down_proj)
    res = run_bass_kernel_spmd(nc, in_maps, list(range(NUM_EXPERTS)))
    return assemble_output(res.results)

